# revision 8
# baseline (speedup 1.0000x reference)
"""2-layer GAT on 8 Trainium2 NeuronCores (Bass/Tile), lane-aligned edition.

Strategy (dst-partitioned, degree-banded windows, lane-per-node):
  * Nodes sorted desc by degree; rank k -> global window k//125 (so each
    window holds 125 nodes of near-identical degree), position k%125.
    Global window g -> core g%8, local window g//8: every core gets 50
    windows spanning the degree spectrum (edge-balanced to ~1%).
  * Within a window, lane p of the 128 SBUF partitions IS dst slot p: all
    edges into that node live on its lane, padded to the window's uniform
    chunk count CW[w] = max degree in the degree band (shared across cores
    so the SPMD program is identical).
  * Phase 1 (sharded + For_i hardware loop): h-table rows
    [h (256 bf16) | a_src (8 f32, bit-cast)] = 544B and the a_dst table
    (8 f32) for this core's 6272 nodes; AllGather both so every core holds
    all nodes ("halo exchange").
  * Layer-1 per window (unrolled; indirect DMAs cannot sit in hardware
    loops on this stack): CW row-gathers of h|a_src by source node (the
    dominant memory-bound traffic), one 128-row gather of a_dst by the
    window's own nodes. Then pure DVE/ACT math: e = a_src + a_dst(lane),
    w = exp(leaky_relu(e)), msg = h * w (4D in-place), and the segment sum
    is just a strided TensorReduce along the chunk axis - the lane IS the
    destination, so no one-hot scatter matmul is needed at all.
  * h1 = elu(numer/denom) reduced against W2 to a scalar h2 per slot
    (kept in a persistent [128, 50] SBUF tile); 200KB AllGather shares h2.
  * Layer-2 per window: CW scalar gathers of h2[src-slot]; h2[dst] is the
    lane's own column of the persistent tile. Same reduce pattern.
  * Pad edges/slots aim at -1e30 pad rows => w = 0.
  * Host preprocessing is fully vectorized; output is inverse-permuted
    from slot order.
"""

import numpy as np
import ml_dtypes

from concourse import bass, mybir
import concourse.tile as tile
from concourse.bass import ds
from concourse.bass_utils import run_bass_kernel_spmd

F32 = mybir.dt.float32
BF16 = mybir.dt.bfloat16
I32 = mybir.dt.int32
AF = mybir.ActivationFunctionType
OP = mybir.AluOpType

N = 50000
IN = 128
HEADS = 8
HID = 32
D = HEADS * HID  # 256
HROW = D + 16  # h-table row: 256 bf16 + 8 f32 (as 16 bf16 slots)
NEG = 0.2
NCORES = 8
P = 128
NEG_BIG = -1.0e30

TPC = 49
ROWS_PC = TPC * P  # 6272
NP2 = NCORES * ROWS_PC  # 50176
N_WIN = 50
NPW = 125  # nodes per window
SLOTS = N_WIN * P  # 6400
WTOT = NCORES * N_WIN  # 400
NSLOT_ALL = NCORES * SLOTS  # 51200
H2EXT = 51328

LAST_EXEC_NS = None
LAST_RESULTS = None


# ---------------------------------------------------------------------------
# walrus workarounds (same as the previous kernels)
def _patch_tile_drain():
    if getattr(tile.TileContext, "_gat_drain_patched", False):
        return

    def _split_drain_and_barrier(self, tick_clock, wait_clock):
        nc = self.nc
        gc = tick_clock.global_clock
        for proc, sem in self.sems.allocated().items():
            tick = gc[proc]
            if tick <= 0:
                continue
            mult = 16 if sem.name.startswith(("DMASW", "DMAHW")) else 1
            nc.sync.nop(nofuse=True).wait_op(sem, tick * mult, "sem-ge")
        nc.sync.drain()
        nc.all_engine_barrier()
        assert self.sems is not None
        popped = nc._tile_sem_poison_stack.pop()
        assert popped is self._sem_poison
        nc.clear_and_free_semaphores(list(self.sems.allocated().values()))
        nc.all_engine_barrier()

    tile.TileContext._drain_and_barrier = _split_drain_and_barrier
    tile.TileContext._gat_drain_patched = True


_WAIT_CAP = 1


def _split_waits_json(bir_json: bytes) -> bytes:
    import json

    m = json.loads(bir_json)
    changed = False
    # Pass 1: expand InstIncSwdgeSem (walrus can't encode it) into per-sem
    # EventSemaphore updates. Only needed if a SWDGE DMA ever lands in a
    # For_i loop; kernel3 keeps indirect DMAs out of loops, so this is a
    # safety net.
    for fn in m.get("functions", []):
        for bb in fn.get("blocks", []):
            insts = bb.get("instructions", [])
            out = []
            for ins in insts:
                if ins.get("op_name") != "InstIncSwdgeSem":
                    out.append(ins)
                    continue
                upd = "sem-add-imm" if ins.get("mode") == "add" else "sem-sub-imm"
                ow = [
                    w
                    for w in (ins.get("sync_info") or {}).get("on_wait") or []
                    if not (
                        w.get("wait_mode") == "sem-ge-imm"
                        and w.get("wait_value", 0) == 0
                    )
                ]
                base = ins["sem_id_base"]
                emitted = False
                for k, (nm, val) in enumerate(
                    zip(ins["sem_names"], ins["sem_values"])
                ):
                    if val == 0:
                        continue
                    out.append(
                        {
                            "debug": ins.get("debug", 0),
                            "engine": ins["engine"],
                            "ins": [],
                            "name": f"{ins['name']}sw{k}",
                            "opcode": "EventSemaphore",
                            "outs": [],
                            "sync_info": {
                                "on_update": [
                                    {
                                        "ant_name": nm,
                                        "id": base + k,
                                        "sync_type": "semaphore",
                                        "update_mode": upd,
                                        "update_value": val,
                                    }
                                ],
                                "on_wait": ow if not emitted else [],
                            },
                        }
                    )
                    emitted = True
                if not emitted and ow:
                    out.append(
                        {
                            "debug": ins.get("debug", 0),
                            "engine": ins["engine"],
                            "ins": [],
                            "name": f"{ins['name']}sw",
                            "opcode": "NoOp",
                            "outs": [],
                            "sync_info": {"on_update": [], "on_wait": ow},
                        }
                    )
                changed = True
            bb["instructions"] = out
    # Pass 2: this walrus build fits at most 2 wait commands per compute
    # instruction; cap at 1 and hoist the rest onto adjacent NoOps.
    for fn in m.get("functions", []):
        for bb in fn.get("blocks", []):
            insts = bb.get("instructions", [])
            out = []
            for ins in insts:
                si = ins.get("sync_info") or {}
                ow = si.get("on_wait") or []
                if len(ow) > _WAIT_CAP:
                    keep = ow[: _WAIT_CAP - 1] if _WAIT_CAP > 1 else []
                    hoist = ow[len(keep) :]
                    keep = keep + [hoist.pop()]
                    for k, w in enumerate(hoist):
                        out.append(
                            {
                                "debug": ins.get("debug", 0),
                                "engine": ins["engine"],
                                "ins": [],
                                "name": f"{ins['name']}w{k}",
                                "opcode": "NoOp",
                                "outs": [],
                                "sync_info": {"on_update": [], "on_wait": [w]},
                            }
                        )
                    si["on_wait"] = keep
                    changed = True
                out.append(ins)
            bb["instructions"] = out
    if not changed:
        return bir_json
    import json as _json

    return _json.dumps(m).encode()


def _scrub_debug_json(bir_json: bytes) -> bytes:
    """Normalize file paths / tracebacks in the BIR so the serialized bytes
    (and hence the neuron compile-cache key and NEFF) are identical no
    matter which directory kernel.py runs from."""
    import json
    import os as _os

    m = json.loads(bir_json)

    def walk(o):
        if isinstance(o, dict):
            if "filename" in o and isinstance(o["filename"], str):
                o["filename"] = _os.path.basename(o["filename"])
            if "ant_traceback" in o:
                o["ant_traceback"] = ""
            for v in o.values():
                walk(v)
        elif isinstance(o, list):
            for v in o:
                walk(v)

    walk(m)
    return json.dumps(m).encode()


def _wrap_scrub(nc):
    orig = nc.to_json_bytes

    def scrubbed():
        return _scrub_debug_json(orig())

    nc.to_json_bytes = scrubbed
    return nc


def _patch_compile_bir():
    import concourse.bass_utils as bu
    import concourse.bass2jax as b2j

    if getattr(bu, "_gat_wait_split_patched", False):
        return
    orig = bu.compile_bir_kernel

    def wrapped(bir_json, tmpdir, neff_name="file.neff"):
        return orig(_split_waits_json(bir_json), tmpdir, neff_name)

    bu.compile_bir_kernel = wrapped
    b2j.compile_bir_kernel = wrapped
    bu._gat_wait_split_patched = True


# ---------------------------------------------------------------------------
# host-side integer preprocessing (fully vectorized)


def preprocess(src, dst):
    n_tot = len(src)
    deg = np.bincount(dst, minlength=N).astype(np.int64)

    order = np.argsort(-deg, kind="stable")
    k = np.arange(N, dtype=np.int64)
    g_of = np.empty(N, np.int64)
    pos_of = np.empty(N, np.int64)
    g_of[order] = k // NPW
    pos_of[order] = k % NPW
    core_of = g_of % NCORES
    win_of = g_of // NCORES
    slot_of = core_of * SLOTS + win_of * P + pos_of

    CW = np.zeros(N_WIN, np.int64)
    np.maximum.at(CW, win_of, deg)
    CW = np.maximum(CW, 1)
    loff = np.zeros(N_WIN + 1, np.int64)
    loff[1:] = np.cumsum(CW * P)
    LTOT = int(loff[-1])

    o = np.argsort(dst, kind="stable")
    es, ed = src[o], dst[o]
    dstart = np.searchsorted(ed, np.arange(N))
    r = np.arange(n_tot, dtype=np.int64) - dstart[ed]
    w_e = win_of[ed]
    flat = loff[w_e] + pos_of[ed] * CW[w_e] + r
    c_e = core_of[ed]

    g1 = np.full((NCORES, LTOT), NP2, np.int64)
    g2 = np.full((NCORES, LTOT), NSLOT_ALL, np.int64)
    g1[c_e, flat] = es
    g2[c_e, flat] = slot_of[es]
    sidx = np.full((NCORES, SLOTS), NP2, np.int64)
    sidx[core_of, win_of * P + pos_of] = np.arange(N, dtype=np.int64)

    return {
        "g1": [np.ascontiguousarray(g1[c]).astype(np.int32) for c in range(NCORES)],
        "g2": [np.ascontiguousarray(g2[c]).astype(np.int32) for c in range(NCORES)],
        "sidx": [
            np.ascontiguousarray(sidx[c]).astype(np.int32).reshape(SLOTS, 1)
            for c in range(NCORES)
        ],
        "CW": [int(c) for c in CW],
        "loff": [int(v) for v in loff],
        "slot_of": slot_of,
        "core_of": core_of,
    }


# ---------------------------------------------------------------------------
# device program


def build_nc(CW, loff, as2, ad2):
    _patch_tile_drain()
    _patch_compile_bir()
    LTOT = loff[-1]

    nc = _wrap_scrub(bass.Bass(disable_frame_to_traceback=True))

    xT = nc.declare_dram_parameter("xT", [IN, ROWS_PC], BF16, isOutput=False)
    w1c = nc.declare_dram_parameter("w1c", [IN, D + 16], BF16, isOutput=False)
    w2rep = nc.declare_dram_parameter("w2rep", [P, D], F32, isOutput=False)
    pads2 = nc.declare_dram_parameter("pads2", [2, 1], F32, isOutput=False)
    g1 = nc.declare_dram_parameter("g1", [LTOT, 1], I32, isOutput=False)
    g2 = nc.declare_dram_parameter("g2", [LTOT, 1], I32, isOutput=False)
    sidx = nc.declare_dram_parameter("sidx", [SLOTS, 1], I32, isOutput=False)
    out2 = nc.declare_dram_parameter("out2", [SLOTS, 1], F32, isOutput=True)

    hA_loc = nc.dram_tensor("hA_loc", [ROWS_PC, HROW], BF16)
    ad_loc = nc.dram_tensor("ad_loc", [ROWS_PC, 8], F32)
    hAfull = nc.dram_tensor("hAfull", [NP2 + 16, HROW], BF16, addr_space="Shared")
    adfull = nc.dram_tensor("adfull", [NP2 + 16, 8], F32, addr_space="Shared")
    h2loc = nc.dram_tensor("h2loc", [SLOTS, 1], F32)
    h2ext = nc.dram_tensor("h2ext", [H2EXT, 1], F32, addr_space="Shared")

    with tile.TileContext(nc) as tc:
        with tc.tile_pool(name="const", bufs=1) as cpool:
            w1c_sb = cpool.tile([IN, D + 16], BF16)
            nc.sync.dma_start(out=w1c_sb[:], in_=w1c[:])
            w2r = cpool.tile([P, D], F32)
            nc.sync.dma_start(out=w2r[:], in_=w2rep[:])
            w2sum = cpool.tile([P, 1], F32)
            nc.vector.reduce_sum(out=w2sum[:], in_=w2r[:], axis=mybir.AxisListType.X)

            # pad rows
            zh = cpool.tile([16, D], BF16)
            nc.gpsimd.memset(zh[:], 0.0)
            nc.sync.dma_start(out=hAfull[NP2 : NP2 + 16, 0:D], in_=zh[:])
            padf = cpool.tile([16, 8], F32)
            nc.gpsimd.memset(padf[:], NEG_BIG)
            nc.sync.dma_start(
                out=hAfull[NP2 : NP2 + 16, D:HROW].bitcast(F32), in_=padf[:]
            )
            nc.sync.dma_start(out=adfull[NP2 : NP2 + 16, :], in_=padf[:])
            p2t = cpool.tile([2, 1], F32)
            nc.sync.dma_start(out=p2t[:], in_=pads2[:])
            nc.sync.dma_start(out=h2ext[NSLOT_ALL : NSLOT_ALL + 2, :], in_=p2t[:])
            zt = cpool.tile([P, 1], F32)
            nc.gpsimd.memset(zt[:], 0.0)
            nc.sync.dma_start(
                out=h2ext[NSLOT_ALL + 2 : H2EXT, :], in_=zt[: H2EXT - NSLOT_ALL - 2]
            )

            # persistent per-slot scalars
            h2sb = cpool.tile([P, N_WIN], F32)
            out2sb = cpool.tile([P, N_WIN], F32)

            # ----- phase 1 (sharded, hardware loop) -----
            with (
                tc.tile_pool(name="p1sb", bufs=3) as p1,
                tc.tile_pool(name="p1ps", bufs=2, space="PSUM") as p1p,
            ):
                with tc.For_i(0, ROWS_PC, P) as ix:
                    xt = p1.tile([IN, P], BF16, tag="xt")
                    nc.sync.dma_start(out=xt[:], in_=xT[:, ds(ix, P)])
                    ph = p1p.tile([P, D + 16], F32)
                    nc.tensor.matmul(
                        out=ph[:], lhsT=xt[:], rhs=w1c_sb[:], start=True, stop=True
                    )
                    hsb = p1.tile([P, HROW], BF16, tag="hsb")
                    nc.scalar.activation(out=hsb[:, 0:D], in_=ph[:, 0:D], func=AF.Copy)
                    nc.vector.tensor_copy(
                        out=hsb[:, D:HROW].bitcast(F32), in_=ph[:, D : D + 8]
                    )
                    asb = p1.tile([P, 8], F32, tag="asb")
                    nc.vector.tensor_copy(out=asb[:], in_=ph[:, D + 8 : D + 16])
                    nc.sync.dma_start(out=hA_loc[ds(ix, P), :], in_=hsb[:])
                    nc.sync.dma_start(out=ad_loc[ds(ix, P), :], in_=asb[:])

            nc.gpsimd.collective_compute(
                "AllGather",
                OP.bypass,
                replica_groups=[list(range(NCORES))],
                ins=[hA_loc[:]],
                outs=[hAfull[0:NP2, :]],
            )
            nc.gpsimd.collective_compute(
                "AllGather",
                OP.bypass,
                replica_groups=[list(range(NCORES))],
                ins=[ad_loc[:]],
                outs=[adfull[0:NP2, :]],
            )

            # ----- layer 1 (unrolled windows) -----
            with (
                tc.tile_pool(name="l1big", bufs=2) as pb,
                tc.tile_pool(name="l1sm", bufs=3) as psm,
            ):
                for w in range(N_WIN):
                    C = CW[w]
                    base = loff[w]
                    idx = psm.tile([P, C], I32, tag="idx")
                    nc.sync.dma_start(
                        out=idx[:],
                        in_=g1[base : base + C * P, :].rearrange(
                            "(p c) o -> p (c o)", p=P
                        ),
                    )
                    sid = psm.tile([P, 1], I32, tag="sid")
                    nc.sync.dma_start(out=sid[:], in_=sidx[w * P : (w + 1) * P, :])

                    hrows = pb.tile([P, C * HROW], BF16, tag="hrows")
                    for j in range(C):
                        nc.gpsimd.indirect_dma_start(
                            out=hrows[:, j * HROW : (j + 1) * HROW],
                            out_offset=None,
                            in_=hAfull[:],
                            in_offset=bass.IndirectOffsetOnAxis(
                                ap=idx[:, j : j + 1], axis=0
                            ),
                        )
                    adr = psm.tile([P, 8], F32, tag="adr")
                    nc.gpsimd.indirect_dma_start(
                        out=adr[:],
                        out_offset=None,
                        in_=adfull[:],
                        in_offset=bass.IndirectOffsetOnAxis(ap=sid[:], axis=0),
                    )

                    hr3 = hrows[:].rearrange("p (c k) -> p c k", k=HROW)
                    asr = hr3[:, :, D:HROW].bitcast(F32)  # [P, C, 8]
                    e_t = psm.tile([P, C * 8], F32, tag="e_t")
                    e3 = e_t[:].rearrange("p (c h) -> p c h", h=8)
                    nc.vector.tensor_tensor(
                        out=e3,
                        in0=asr,
                        in1=adr[:, None, :].to_broadcast([P, C, 8]),
                        op=OP.add,
                    )
                    lr = psm.tile([P, C * 8], F32, tag="lr")
                    nc.vector.tensor_scalar_mul(lr[:], e_t[:], NEG)
                    nc.vector.tensor_tensor(
                        out=lr[:], in0=lr[:], in1=e_t[:], op=OP.max
                    )
                    w_t = psm.tile([P, C * 8], F32, tag="w_t")
                    nc.scalar.activation(out=w_t[:], in_=lr[:], func=AF.Exp)

                    # msg = h * w, in place (4D broadcast over the 32 feats)
                    h4 = hr3[:, :, 0:D].rearrange("p c (h x) -> p c h x", h=HEADS)
                    w4 = (
                        w_t[:]
                        .rearrange("p (c h) -> p c h", h=8)[:, :, :, None]
                        .to_broadcast([P, C, 8, HID])
                    )
                    nc.vector.tensor_tensor(out=h4, in0=h4, in1=w4, op=OP.mult)

                    numer = psm.tile([P, D], F32, tag="numer")
                    nc.vector.reduce_sum(
                        out=numer[:],
                        in_=hr3[:, :, 0:D].rearrange("p c k -> p k c"),
                        axis=mybir.AxisListType.X,
                    )
                    denom = psm.tile([P, 8], F32, tag="denom")
                    nc.vector.reduce_sum(
                        out=denom[:],
                        in_=w_t[:].rearrange("p (c h) -> p h c", h=8),
                        axis=mybir.AxisListType.X,
                    )
                    nc.vector.tensor_scalar_max(denom[:], denom[:], 1e-30)
                    rcp = psm.tile([P, 8], F32, tag="rcp")
                    nc.vector.reciprocal(rcp[:], denom[:])
                    o1 = psm.tile([P, D], F32, tag="o1")
                    nc.vector.tensor_tensor(
                        out=o1[:].rearrange("p (h x) -> p h x", h=HEADS),
                        in0=numer[:].rearrange("p (h x) -> p h x", h=HEADS),
                        in1=rcp[:][:, :, None].to_broadcast([P, HEADS, HID]),
                        op=OP.mult,
                    )
                    # elu+1 = max(o1,0) + exp(min(o1,0))
                    mn = psm.tile([P, D], F32, tag="mn")
                    nc.vector.tensor_scalar_min(mn[:], o1[:], 0.0)
                    ex = psm.tile([P, D], F32, tag="ex")
                    nc.scalar.activation(out=ex[:], in_=mn[:], func=AF.Exp)
                    nc.vector.tensor_scalar_max(o1[:], o1[:], 0.0)
                    s1 = psm.tile([P, D], F32, tag="s1")
                    nc.vector.tensor_tensor(out=s1[:], in0=o1[:], in1=ex[:], op=OP.add)
                    nc.vector.tensor_tensor(
                        out=s1[:], in0=s1[:], in1=w2r[:], op=OP.mult
                    )
                    nc.vector.reduce_sum(
                        out=h2sb[:, w : w + 1], in_=s1[:], axis=mybir.AxisListType.X
                    )
                    nc.vector.tensor_scalar(
                        out=h2sb[:, w : w + 1],
                        in0=h2sb[:, w : w + 1],
                        scalar1=w2sum[:],
                        scalar2=None,
                        op0=OP.subtract,
                    )

                nc.sync.dma_start(
                    out=h2loc[:].rearrange("(w p) o -> p (w o)", p=P), in_=h2sb[:]
                )

            nc.gpsimd.collective_compute(
                "AllGather",
                OP.bypass,
                replica_groups=[list(range(NCORES))],
                ins=[h2loc[:]],
                outs=[h2ext[0:NSLOT_ALL, :]],
            )

            # ----- layer 2 (unrolled windows) -----
            with tc.tile_pool(name="l2sm", bufs=3) as p4:
                for w in range(N_WIN):
                    C = CW[w]
                    base = loff[w]
                    idx2 = p4.tile([P, C], I32, tag="idx2")
                    nc.sync.dma_start(
                        out=idx2[:],
                        in_=g2[base : base + C * P, :].rearrange(
                            "(p c) o -> p (c o)", p=P
                        ),
                    )
                    gv = p4.tile([P, C], F32, tag="gv")
                    for j in range(C):
                        nc.gpsimd.indirect_dma_start(
                            out=gv[:, j : j + 1],
                            out_offset=None,
                            in_=h2ext[:],
                            in_offset=bass.IndirectOffsetOnAxis(
                                ap=idx2[:, j : j + 1], axis=0
                            ),
                        )
                    e2 = p4.tile([P, C], F32, tag="e2")
                    nc.vector.tensor_scalar_mul(e2[:], gv[:], float(as2))
                    hd = p4.tile([P, 1], F32, tag="hd")
                    nc.vector.tensor_scalar_mul(hd[:], h2sb[:, w : w + 1], float(ad2))
                    nc.vector.tensor_tensor(
                        out=e2[:],
                        in0=e2[:],
                        in1=hd[:].to_broadcast([P, C]),
                        op=OP.add,
                    )
                    lr2 = p4.tile([P, C], F32, tag="lr2")
                    nc.vector.tensor_scalar_mul(lr2[:], e2[:], NEG)
                    nc.vector.tensor_tensor(
                        out=lr2[:], in0=lr2[:], in1=e2[:], op=OP.max
                    )
                    w2t = p4.tile([P, C], F32, tag="w2t")
                    nc.scalar.activation(out=w2t[:], in_=lr2[:], func=AF.Exp)
                    m2 = p4.tile([P, C], F32, tag="m2")
                    nc.vector.tensor_tensor(
                        out=m2[:], in0=w2t[:], in1=gv[:], op=OP.mult
                    )
                    den = p4.tile([P, 1], F32, tag="den")
                    nc.vector.reduce_sum(
                        out=den[:], in_=w2t[:], axis=mybir.AxisListType.X
                    )
                    nc.vector.tensor_scalar_max(den[:], den[:], 1e-30)
                    r2 = p4.tile([P, 1], F32, tag="r2")
                    nc.vector.reciprocal(r2[:], den[:])
                    num = p4.tile([P, 1], F32, tag="num")
                    nc.vector.reduce_sum(
                        out=num[:], in_=m2[:], axis=mybir.AxisListType.X
                    )
                    nc.vector.tensor_tensor(
                        out=out2sb[:, w : w + 1], in0=num[:], in1=r2[:], op=OP.mult
                    )

                nc.sync.dma_start(
                    out=out2[:].rearrange("(w p) o -> p (w o)", p=P), in_=out2sb[:]
                )

    return nc


# ---------------------------------------------------------------------------
# top-level entry


def kernel(x, edge_index, W1, att_src1, att_dst1, b1, W2, att_src2, att_dst2, b2):
    global LAST_EXEC_NS, LAST_RESULTS

    import sys as _sys
    import time as _time

    _tstage = _time.monotonic()

    def _stamp(msg):
        nonlocal _tstage
        now = _time.monotonic()
        print(
            f"[kernel-stage] {msg}: {now - _tstage:.2f}s", file=_sys.stderr, flush=True
        )
        _tstage = now

    x = np.asarray(x, np.float32)
    edge_index = np.asarray(edge_index).astype(np.int64)
    W1 = np.asarray(W1, np.float32)
    att_src1 = np.asarray(att_src1, np.float32)
    att_dst1 = np.asarray(att_dst1, np.float32)
    b1 = np.asarray(b1, np.float32)
    W2 = np.asarray(W2, np.float32)
    as2 = float(np.asarray(att_src2).reshape(-1)[0])
    ad2 = float(np.asarray(att_dst2).reshape(-1)[0])
    b2 = np.asarray(b2, np.float32)
    assert not (as2 == 0.0 and ad2 == 0.0)
    assert np.all(b1 == 0) and np.all(b2 == 0), "nonzero biases not folded"

    loops = np.arange(N, dtype=np.int64)
    src = np.concatenate([edge_index[0], loops])
    dst = np.concatenate([edge_index[1], loops])

    pp = preprocess(src, dst)
    _stamp("preprocess")

    W1r = W1.reshape(IN, HEADS, HID)
    ws1 = (W1r * att_src1[None]).sum(-1)
    wd1 = (W1r * att_dst1[None]).sum(-1)
    w1cat = np.concatenate([W1, ws1, wd1], axis=1).astype(ml_dtypes.bfloat16)

    w2rep = np.repeat(W2.reshape(1, D), P, axis=0).astype(np.float32)
    pads2 = np.array(
        [
            [NEG_BIG * np.sign(as2) if as2 != 0 else 0.0],
            [NEG_BIG * np.sign(ad2) if ad2 != 0 else 0.0],
        ],
        np.float32,
    )

    x_pad = np.zeros((NP2, IN), np.float32)
    x_pad[:N] = x
    xT_bf = np.ascontiguousarray(x_pad.T).astype(ml_dtypes.bfloat16)

    nc = build_nc(pp["CW"], pp["loff"], as2, ad2)
    _stamp("build_nc")

    in_maps = []
    for c in range(NCORES):
        in_maps.append(
            {
                "xT": np.ascontiguousarray(xT_bf[:, c * ROWS_PC : (c + 1) * ROWS_PC]),
                "w1c": w1cat,
                "w2rep": w2rep,
                "pads2": pads2,
                "g1": pp["g1"][c].reshape(-1, 1),
                "g2": pp["g2"][c].reshape(-1, 1),
                "sidx": pp["sidx"][c],
            }
        )

    _t0 = _time.monotonic()
    res = run_bass_kernel_spmd(nc, in_maps, core_ids=list(range(NCORES)))
    _wall_ns = int((_time.monotonic() - _t0) * 1e9)
    _stamp("run_bass_kernel_spmd")
    LAST_RESULTS = res
    LAST_EXEC_NS = res.exec_time_ns if res.exec_time_ns is not None else _wall_ns

    out = np.empty(N, np.float32)
    slot_of = pp["slot_of"]
    core_of = pp["core_of"]
    for c in range(NCORES):
        m = core_of == c
        out[m] = res.results[c]["out2"].reshape(-1)[slot_of[m] - c * SLOTS]
    return out
